# revision 37
# baseline (speedup 1.0000x reference)
"""Trainium2 Bass kernel: gumbel-softmax-argmax embedding lookup (end-to-end).

Reference math (nn_End2End_49495203119139):
    hot  = argmax_V(softmax((logits + gumbel)/tau))  == argmax_V(logits+gumbel)
    row  = grid_sample-nearest index map of hot      == ROWMAP[hot]  (LUT)
    inputs_embeds = W[row] * mask          (col map == arange(E), verified)
    psg branch: roll/flip/rotate of psg ids, flag = cumsum(trunc!=0) > 0,
    out = inputs_embeds + where(flag, W[trunc], 0)

Key structural fact (verified exactly in numpy): the attention mask is a
contiguous run of len_b ones then zeros, which makes the two branches
DISJOINT per position:
    l <  len_b: out[b,l] = W[ROWMAP[argmax_V(logits+gumbel)[b,l]]]
    l >= len_b: out[b,l] = W[psg_roll[b, l-len_b]],
                psg_roll[0] = 1 (BOS), psg_roll[j] = psg[b, j-1]
so only the sum(len_b) ACTIVE positions need the memory-bound vocab stream.
For the canonical input that is 1419/2048 positions (69%).

Sharding: the active positions are resharded EVENLY across the 8 cores
(Ta = ceil(A/8) vocab-streams per core), which makes the per-core HBM
traffic ~2*Ta*128.5KB + small gathers.  Each active token's 32128-float
logits row is folded host-side to [128, 251] (partition-major), and a
core's Ta tokens are concatenated along the free axis -> DRAM
[128*Ta, 251] viewed as [128, Ta*251], so every streaming DMA is a
full-width 128-partition transfer.

Device plan per core.  The cost model serializes ALL DMA transfers on one
exclusive DMA-engine pool at 360 GB/s, so sim time decomposes as
start-latency + stream-phase + end-chain; the kernel minimizes each:
  - stream phase: chunks of tpc tokens = [128, tpc*251] f32. HWDGE(sync
    ring) loads the logits chunk; SWDGE CCE-add DMA accumulates the gumbel
    chunk in the DMA datapath; ONE 3D-AP DVE reduce_max per chunk ->
    per-token strip maxes into the block stats tile mx[128, 128].
  - CHUNK ORDER (ILV): the last block's pre-final chunks stream FIRST,
    then the other blocks, then the FINAL granule's chunks - so every
    other granule completes far from the stream end and its tail chain
    resolves under later streaming.
  - granule tails (64 tokens): ph0 32x32 stream-transposes -> token-major
    mxTg; max/max_index -> winning strip p*; ph1a two indirect gathers
    re-fetch the winning [1,251] logits/gumbel strips; ph1b DVE add +
    max/max_index -> within-strip c*; hot = p**251 + c*.  Phase emission
    dues are spread so the in-order SWDGE/DVE queues never stall streaming
    and rescaled into the remaining-chunk window when a granule completes
    late.
  - DEFERRAL: every granule's W-row gather (ph2, from wrm = W[rowmap]
    precomposed host-side) and store (ph3), plus the whole psg branch's
    gather+store, run at the END, inside the final resolve's DMA-idle
    window - removing ~4.5us of DMA from the stream phase for free.
    pidx loads happen at start; hot_i tiles persist until the flush.
  - FINAL granule (FT tokens, default 16): its chunks additionally track
    the within-strip argmax index in-stream (per-token DVE max_index into
    ix), so its post-stream resolve needs NO strip re-gather: transpose ->
    p* -> integer-only select of c* from ix (3 DVE ops) -> one W gather ->
    one store on the idle sync ring.
  - variant hooks kept for experiments: LAD/POOL_LADDER (a small tracked
    tail granule per non-last block resolved via gpsimd
    partition_all_reduce - sims ~1us faster but the Q7 library mix
    (iota/standard + DMAGatherAnt/mlp + PartitionAllReduce/attn) fails
    NEFF compile, so it is OFF), POOL_RESOLVE, IDX_MODE, TLB, TPC, GS.

Host does only sharding/unsharding: selecting + reordering rows by the mask
(data-dependent sharding), the [128,251] fold, and scattering the returned
rows into [16,128,768]. All arithmetic on tensor VALUES happens on device.

Tie-breaking matches jnp argmax (first occurrence) exactly: vocab index =
p*251+c with strips in vocab order; DVE max_index returns the first index;
cross-partition winner is the first partition attaining the max; CCE f32
add == DVE f32 add bitwise, so streamed maxes and re-gathered strips agree.
"""

import numpy as np

B = 16
L = 128
V = 32128
E = 768
N_CORES = 8
P = 128                   # partitions; V = P * C
C = V // P                # 251 columns per folded strip
TPC = 8                   # accum-split cap: 8 tokens -> 2008 elems <= 2048
TPC_CHUNK = 2             # tokens per streamed chunk
NEG = -3.0e38


def _build(nc_mod, ta, tp, variant=None):
    """Build the per-core Bass module for ta active streams + tp psg rows."""
    import concourse.tile as tile
    from concourse import bass, bass_isa, mybir
    from concourse.bass import IndirectOffsetOnAxis

    var = variant or {}

    nc = nc_mod
    f32 = mybir.dt.float32
    i32 = mybir.dt.int32
    u32 = mybir.dt.uint32
    Op = mybir.AluOpType
    AX = mybir.AxisListType

    n_blk = (ta + P - 1) // P

    # logits fold rows [0, P*ta), gumbel fold rows [P*ta, 2*P*ta): one tensor
    # so the strip re-gathers share a single gather source
    lgg_h = nc.dram_tensor("lgg", [2 * P * ta, C], f32, kind="ExternalInput")
    # wrm = W[rowmap] precomposed host-side (rowmap is a pure LUT of V)
    wrm_h = nc.dram_tensor("wrm", [V, E], f32, kind="ExternalInput")
    out_act_h = nc.dram_tensor("out_act", [ta, E], f32, kind="ExternalOutput")
    if tp:
        wte_h = nc.dram_tensor("wte", [V, E], f32, kind="ExternalInput")
        psgidx_h = nc.dram_tensor("psgidx", [tp, 1], i32, kind="ExternalInput")
        out_psg_h = nc.dram_tensor("out_psg", [tp, E], f32, kind="ExternalOutput")

    # [128, ta*251] streaming views (row-major contiguous reshapes)
    lg2 = lgg_h[0:P * ta, :].rearrange("(p t) c -> p (t c)", p=P)
    gm2 = lgg_h[P * ta:2 * P * ta, :].rearrange("(p t) c -> p (t c)", p=P)

    S = 32                              # transpose block size
    GS = var.get("GS", 64)              # granule: tokens per tail set
    idx_mode = var.get("IDX_MODE", False)
    tlb = var.get("TLB", False)         # track the whole last block
    lgs = var.get("LAST_GS", GS)        # granule size within the last block
    ft_cfg = var.get("FT", 16)          # idx-tracked final-granule size
    psg_end = var.get("PSG_END", True)
    reserve = var.get("RESERVE", 0)     # chunks kept clear of late-granule
                                        # phase emission at the stream end
    int_idx = var.get("INT_IDX", True)  # integer-only tracked resolve
    defer = var.get("DEFER", True)      # W gathers/stores in the end window
    pool_res = var.get("POOL_RESOLVE", False)  # granule resolve on the Pool
    # engine via partition_all_reduce (no transposes, no DVE tail work)

    def ceil32(x):
        return -(-x // S) * S

    # ---- per-block granule plan: list of (lo, hi, tracked) ----
    # tracked granules resolve from in-stream argmax indices (no re-gather);
    # the LAST block's final granule is small (FT) and tracked so the
    # post-stream chain is short.
    lad = var.get("LAD", 0)   # tracked tail-ladder granule per non-last block
    blk_plan = []
    blk_trk_base = []         # ix tile base column, or None
    blk_trk_wlo = []          # first token whose index is tracked in-stream
    blk_ft_lo = []            # ladder start: tail chunks streamed at the end
    for b in range(n_blk):
        nt = min(P, ta - b * P)
        last = b == n_blk - 1
        track_all = idx_mode or pool_res or (tlb and last)
        # tail granule: FT for the last block (streamed at the very end),
        # LAD for the others (streamed just before it) - small and tracked
        # so no re-gather chain can land in the final stream window
        f_cfg = ft_cfg if last else lad
        f = min(f_cfg, nt) if f_cfg else 0
        if f and f >= nt and nt > S:
            f = 0 if last else 0
        plan = []
        lo = 0
        edge = nt - f
        gsz = lgs if last else GS
        while lo < edge:
            hi = min(lo + gsz, edge)
            plan.append((lo, hi, track_all))
            lo = hi
        if f:
            plan.append((edge, nt, True))
        blk_plan.append(plan)
        blk_ft_lo.append(edge if f else nt)
        # tracked-granule transpose windows may shift down when lo+gt > P;
        # the ix tile must start at the lowest shifted window base
        bases = []
        wlos = []
        for lo2, hi2, tr in plan:
            if tr:
                gt2 = ceil32(hi2 - lo2)
                bases.append(lo2 if lo2 + gt2 <= P else P - gt2)
                wlos.append(lo2)
        blk_trk_base.append(min(bases) if bases else None)
        blk_trk_wlo.append(min(wlos) if wlos else None)

    with tile.TileContext(nc) as tc:
        with (
            tc.tile_pool(name="lpool", bufs=var.get("LBUFS", 12)) as lpool,
            tc.tile_pool(name="stats", bufs=4 * n_blk) as stats,
            tc.tile_pool(name="small", bufs=3) as small,
            tc.tile_pool(name="emb", bufs=2) as emb,
            tc.tile_pool(name="wide", bufs=var.get("WBUFS", 6)) as wide,
            tc.tile_pool(name="consts", bufs=1) as consts,
        ):
            # ---- constants ----
            iota_p_i = consts.tile([P, 1], i32)
            nc.gpsimd.iota(iota_p_i[:], pattern=[[1, 1]], base=0, channel_multiplier=1)
            iota_pf = consts.tile([P, 1], f32)
            nc.vector.tensor_copy(out=iota_pf[:], in_=iota_p_i[:])
            revp = eyef = None
            if pool_res or (lad and var.get("POOL_LADDER", True) and n_blk > 1):
                # revp = 128 - p (first-occurrence tie-break via max);
                # eyef = identity mask for the diagonal extraction
                revp = consts.tile([P, 1], f32)
                nc.vector.tensor_scalar(revp[:], iota_pf[:], -1.0, float(P),
                                        op0=Op.mult, op1=Op.add)
                iota_c_e = consts.tile([P, P], i32, tag="iota_c_e")
                nc.gpsimd.iota(iota_c_e[:], pattern=[[1, P]], base=0,
                               channel_multiplier=0)
                iota_ce_f = consts.tile([P, P], f32, tag="iota_ce_f")
                nc.vector.tensor_copy(out=iota_ce_f[:], in_=iota_c_e[:])
                ones_pp = consts.tile([P, P], f32, tag="ones_pp")
                nc.vector.memset(ones_pp[:], 1.0)
                eyef = consts.tile([P, P], f32, tag="eyef")
                nc.vector.scalar_tensor_tensor(
                    out=eyef[:], in0=iota_ce_f[:], scalar=iota_pf[:, 0:1],
                    in1=ones_pp[:], op0=Op.is_equal, op1=Op.mult)
            iota_cols = iota_cols_u = None
            if any(t is not None for t in blk_trk_base):
                # per-partition row 0..127 along the free axis (strip ids)
                if int_idx:
                    iota_cols_u = consts.tile([P, P], u32)
                    nc.gpsimd.iota(iota_cols_u[:], pattern=[[1, P]], base=0,
                                   channel_multiplier=0)
                else:
                    iota_c_i = consts.tile([P, P], i32)
                    nc.gpsimd.iota(iota_c_i[:], pattern=[[1, P]], base=0,
                                   channel_multiplier=0)
                    iota_cols = consts.tile([P, P], f32)
                    nc.vector.tensor_copy(out=iota_cols[:], in_=iota_c_i[:])

            psg_tiles = []

            def psg_load():
                # pidx loads only (tiny); gather+store deferred to psg_tail
                for p0 in range(0, tp, P):
                    pn = min(P, tp - p0)
                    pidx = consts.tile([pn, 1], i32, tag=f"pidx{p0}")
                    nc.scalar.dma_start(out=pidx[:], in_=psgidx_h[p0:p0 + pn, :])
                    psg_tiles.append((p0, pn, pidx))

            psg_embs = []

            def psg_gather():
                # pure indirect W gather; runs in the end window where the
                # DMA pool is otherwise idle
                for p0, pn, pidx in psg_tiles:
                    pemb = wide.tile([pn, E], f32, tag="pemb")
                    nc.gpsimd.indirect_dma_start(
                        out=pemb[:], out_offset=None, in_=wte_h[:],
                        in_offset=IndirectOffsetOnAxis(ap=pidx[:, 0:1], axis=0))
                    psg_embs.append((p0, pn, pemb))

            def psg_store():
                for p0, pn, pemb in psg_embs:
                    nc.scalar.dma_start(out=out_psg_h[p0:p0 + pn, :], in_=pemb[:])

            def psg_phase():
                # non-deferred fallback: load + gather + store up front
                for p0 in range(0, tp, P):
                    pn = min(P, tp - p0)
                    pidx = small.tile([pn, 1], i32, tag="pidx")
                    nc.scalar.dma_start(out=pidx[:], in_=psgidx_h[p0:p0 + pn, :])
                    pemb = emb.tile([pn, E], f32, tag="pemb")
                    nc.gpsimd.indirect_dma_start(
                        out=pemb[:], out_offset=None, in_=wte_h[:],
                        in_offset=IndirectOffsetOnAxis(ap=pidx[:, 0:1], axis=0))
                    nc.scalar.dma_start(out=out_psg_h[p0:p0 + pn, :], in_=pemb[:])

            def granule_phases(b, lo, hi, tracked, mx, ix, trk_base,
                               use_pool=False):
                """Phases resolving tokens [b*128+lo, b*128+hi): each later
                phase's work depends only on phases issued earlier, so the
                in-order SWDGE/DVE queues never stall the streaming on tail
                dependencies."""
                t0b = b * P
                gs = hi - lo                     # real tokens
                gt = ceil32(gs)                  # padded transpose extent
                # shift the transpose window down when it would overrun the
                # 128-col stats tile; rows [ro, ro+gs) of the resolve then
                # hold this granule's tokens (lower rows redundantly resolve
                # earlier tokens - harmless)
                lo2 = lo if lo + gt <= P else P - gt
                ro = lo - lo2
                assert ro == 0 or tracked
                st = {}

                def ph0():
                    # transpose mx[:, lo:lo+gt] into a base-0 token-major
                    # tile (exact 32x32 copies) and resolve p* per token.
                    # All compute tiles sit at partition base 0: the BIR
                    # verifier requires equal base partitions for two-SB-input
                    # instructions (NCC_IBIR297).
                    mxTg = stats.tile([gt, P], f32, tag="mxTg")
                    for j in range(gt // S):
                        for i in range(P // S):
                            nc.vector.transpose(
                                out=mxTg[S * j:S * (j + 1), S * i:S * i + S],
                                in_=mx[S * i:S * i + S,
                                       lo2 + S * j:lo2 + S * (j + 1)])
                    if tracked:
                        # transpose the strided col-0-of-8 view of ix
                        ix3 = ix[:, :].rearrange("p (t e) -> p t e", e=8)
                        ixTg = stats.tile([gt, P], u32, tag="ixTg")
                        off = lo2 - trk_base
                        for j in range(gt // S):
                            for i in range(P // S):
                                nc.vector.transpose(
                                    out=ixTg[S * j:S * (j + 1), S * i:S * i + S],
                                    in_=ix3[S * i:S * i + S,
                                            off + S * j:off + S * (j + 1), 0])
                        st["ixTg"] = ixTg
                    gmax8 = small.tile([gt, 8], f32, tag="gmax8")
                    nc.vector.max(out=gmax8[:], in_=mxTg[:])
                    p8 = small.tile([gt, 8], u32, tag="p8")
                    nc.vector.max_index(out=p8[:], in_max=gmax8[:], in_values=mxTg[:])
                    st["p8"] = p8
                    if not (tracked and int_idx):
                        p1f = small.tile([gt, 1], f32, tag="p1f")
                        nc.vector.tensor_copy(out=p1f[:], in_=p8[:, 0:1])
                        st["p1f"] = p1f
                    if not tracked:
                        # fold rows of the winning strips in lgg: col0 =
                        # logits half (p*ta + t), col1 = gumbel half (+ P*ta)
                        tofs = small.tile([gt, 1], f32, tag="tofs")
                        nc.vector.tensor_scalar(tofs[:], iota_pf[0:gt],
                                                float(t0b + lo2), None, op0=Op.add)
                        rows2 = small.tile([gt, 2], f32, tag="rows2")
                        nc.vector.scalar_tensor_tensor(
                            out=rows2[:, 0:1], in0=p1f[:], scalar=float(ta),
                            in1=tofs[:], op0=Op.mult, op1=Op.add)
                        nc.vector.tensor_scalar(rows2[:, 1:2], rows2[:, 0:1],
                                                float(P * ta), None, op0=Op.add)
                        rows2i = small.tile([gt, 2], i32, tag="rows2i")
                        nc.vector.tensor_copy(out=rows2i[:], in_=rows2[:])
                        st["rows2i"] = rows2i

                def ph0_idx():
                    # c* from the in-stream index stats: select column p* of
                    # ixTg (one masked multiply + reduce), no strip re-fetch.
                    # Integer path: 3 ops on the critical end chain instead
                    # of 5 (no f32 round trips).
                    if int_idx:
                        p1u = small.tile([gt, 1], u32, tag="p1u")
                        nc.vector.tensor_copy(out=p1u[:], in_=st["p8"][:, 0:1])
                        selu = small.tile([gt, P], u32, tag="selu")
                        nc.vector.scalar_tensor_tensor(
                            out=selu[:], in0=iota_cols_u[0:gt, :],
                            scalar=p1u[:, 0:1], in1=st["ixTg"][:],
                            op0=Op.is_equal, op1=Op.mult)
                        c1u = small.tile([gt, 1], u32, tag="c1u")
                        nc.vector.reduce_max(out=c1u[:], in_=selu[:], axis=AX.X)
                        hot_i = consts.tile([gt, 1], i32, tag=f"hot{b}_{lo}")
                        nc.vector.scalar_tensor_tensor(
                            out=hot_i[:], in0=p1u[:], scalar=float(C),
                            in1=c1u[:], op0=Op.mult, op1=Op.add)
                        st["hot_i"] = hot_i
                        return
                    ixTf = small.tile([gt, P], f32, tag="ixTf")
                    nc.vector.tensor_copy(out=ixTf[:], in_=st["ixTg"][:])
                    selx = small.tile([gt, P], f32, tag="selx")
                    nc.vector.scalar_tensor_tensor(
                        out=selx[:], in0=iota_cols[0:gt, :],
                        scalar=st["p1f"][:, 0:1], in1=ixTf[:],
                        op0=Op.is_equal, op1=Op.mult)
                    c1f = small.tile([gt, 1], f32, tag="c1f")
                    nc.vector.reduce_max(out=c1f[:], in_=selx[:], axis=AX.X)
                    hotf = small.tile([gt, 1], f32, tag="hotf")
                    nc.vector.scalar_tensor_tensor(
                        out=hotf[:], in0=st["p1f"], scalar=float(C), in1=c1f[:],
                        op0=Op.mult, op1=Op.add)
                    hot_i = consts.tile([gt, 1], i32, tag=f"hot{b}_{lo}")
                    nc.vector.tensor_copy(out=hot_i[:], in_=hotf[:])
                    st["hot_i"] = hot_i

                def ph1a():
                    # two single-index gathers fetch the winning logits and
                    # gumbel strips. (A fused 2-index gather simulates
                    # per-index in CoreSim, but HW ignores the second index -
                    # probed on device; keep them separate.)
                    stl = emb.tile([gt, 2 * C], f32, tag="stl")
                    nc.gpsimd.indirect_dma_start(
                        out=stl[:, 0:C], out_offset=None, in_=lgg_h[:],
                        in_offset=IndirectOffsetOnAxis(
                            ap=st["rows2i"][:, 0:1], axis=0))
                    nc.gpsimd.indirect_dma_start(
                        out=stl[:, C:2 * C], out_offset=None, in_=lgg_h[:],
                        in_offset=IndirectOffsetOnAxis(
                            ap=st["rows2i"][:, 1:2], axis=0))
                    st["stl"] = stl

                def ph1b():
                    # recompute l+g on the fetched strips, find c*
                    stl = st["stl"]
                    ssum = emb.tile([gt, C], f32, tag="ssum")
                    nc.vector.tensor_tensor(out=ssum[:], in0=stl[:, 0:C],
                                            in1=stl[:, C:2 * C], op=Op.add)
                    s8 = small.tile([gt, 8], f32, tag="s8")
                    nc.vector.max(out=s8[:], in_=ssum[:])
                    c8 = small.tile([gt, 8], u32, tag="c8")
                    nc.vector.max_index(out=c8[:], in_max=s8[:], in_values=ssum[:])
                    c1f = small.tile([gt, 1], f32, tag="c1f")
                    nc.vector.tensor_copy(out=c1f[:], in_=c8[:, 0:1])
                    hotf = small.tile([gt, 1], f32, tag="hotf")
                    nc.vector.scalar_tensor_tensor(
                        out=hotf[:], in0=st["p1f"], scalar=float(C), in1=c1f[:],
                        op0=Op.mult, op1=Op.add)
                    hot_i = consts.tile([gt, 1], i32, tag=f"hot{b}_{lo}")
                    nc.vector.tensor_copy(out=hot_i[:], in_=hotf[:])
                    st["hot_i"] = hot_i

                def ph2():
                    # W[rowmap[.]] is precomposed host-side into wrm; gather
                    # only the real rows (pad rows would gather row 0).
                    # Single-index gathers are unsupported - floor at 2.
                    ro2 = st.get("hot_ro", ro)
                    gn = min(max(2, gs), gt - ro2)
                    wrows = wide.tile([gn, E], f32, tag="wrows")
                    nc.gpsimd.indirect_dma_start(
                        out=wrows[:], out_offset=None, in_=wrm_h[:],
                        in_offset=IndirectOffsetOnAxis(
                            ap=st["hot_i"][ro2:ro2 + gn, 0:1], axis=0))
                    st["wrows"] = wrows

                def ph3():
                    # scalar ring mid-stream (never stalls chunk loads on
                    # sync); the FINAL store takes the sync ring, which is
                    # idle at the stream end, so it does not queue behind
                    # the deferred stores draining on the scalar ring.
                    final = tracked and hi == min(P, ta - t0b) and b == n_blk - 1
                    eng = nc.sync if (final and var.get("FT_SYNC_STORE", True)) \
                        else nc.scalar
                    eng.dma_start(out=out_act_h[t0b + lo:t0b + hi, :],
                                  in_=st["wrows"][0:gs, :])

                def pp0():
                    # cross-partition max + first-winner key, all on Pool
                    pam = stats.tile([P, gs], f32, tag="pam")
                    nc.gpsimd.partition_all_reduce(
                        pam[:], mx[:, lo:hi], channels=P,
                        reduce_op=bass_isa.ReduceOp.max)
                    m = stats.tile([P, gs], f32, tag="pm")
                    nc.gpsimd.tensor_tensor(out=m[:], in0=mx[:, lo:hi],
                                            in1=pam[:], op=Op.is_equal)
                    keyp = stats.tile([P, gs], f32, tag="keyp")
                    nc.gpsimd.tensor_tensor(
                        out=keyp[:], in0=m[:],
                        in1=revp[:, 0:1].to_broadcast([P, gs]), op=Op.mult)
                    st["keyp"] = keyp

                def pp1():
                    pk = stats.tile([P, gs], f32, tag="pk")
                    nc.gpsimd.partition_all_reduce(
                        pk[:], st["keyp"][:], channels=P,
                        reduce_op=bass_isa.ReduceOp.max)
                    m2 = stats.tile([P, gs], f32, tag="pm2")
                    nc.gpsimd.tensor_tensor(out=m2[:], in0=st["keyp"][:],
                                            in1=pk[:], op=Op.is_equal)
                    st["pk"], st["m2"] = pk, m2

                def pp2():
                    # select the winning strip's tracked in-strip index
                    ix3 = ix[:, :].rearrange("p (t e) -> p t e", e=8)
                    ix0f = stats.tile([P, gs], f32, tag="ix0f")
                    nc.gpsimd.tensor_copy(
                        out=ix0f[:],
                        in_=ix3[:, lo - trk_base:hi - trk_base, 0])
                    selc = stats.tile([P, gs], f32, tag="selc")
                    nc.gpsimd.tensor_tensor(out=selc[:], in0=st["m2"][:],
                                            in1=ix0f[:], op=Op.mult)
                    st["selc"] = selc

                def pp3():
                    c2 = stats.tile([P, gs], f32, tag="c2")
                    nc.gpsimd.partition_all_reduce(
                        c2[:], st["selc"][:], channels=P,
                        reduce_op=bass_isa.ReduceOp.max)
                    # hot = (128-pk)*251 + c = 32128 - 251*pk + c
                    hotf = stats.tile([P, gs], f32, tag="photf")
                    nc.gpsimd.scalar_tensor_tensor(
                        out=hotf[:], in0=st["pk"][:], scalar=-float(C),
                        in1=c2[:], op0=Op.mult, op1=Op.add)
                    hotf2 = stats.tile([P, gs], f32, tag="photf2")
                    nc.gpsimd.tensor_scalar(hotf2[:], hotf[:], float(V), None,
                                            op0=Op.add)
                    st["hotf2"] = hotf2

                def pp4():
                    # diagonal extraction: hot is replicated across
                    # partitions, so eye-mask + free-axis reduce yields the
                    # partition-major offset column for the gather
                    hd = stats.tile([gs, gs], f32, tag="phd")
                    nc.gpsimd.tensor_tensor(out=hd[:],
                                            in0=st["hotf2"][0:gs, :],
                                            in1=eyef[0:gs, 0:gs], op=Op.mult)
                    hcf = small.tile([gs, 1], f32, tag="phcf")
                    nc.vector.reduce_max(out=hcf[:], in_=hd[:], axis=AX.X)
                    hot_i = consts.tile([gs, 1], i32, tag=f"hot{b}_{lo}")
                    nc.vector.tensor_copy(out=hot_i[:], in_=hcf[:])
                    st["hot_i"] = hot_i
                    st["hot_ro"] = 0

                if (pool_res or use_pool) and tracked:
                    pre = [pp0, pp1, pp2, pp3, pp4]
                elif tracked:
                    pre = [ph0, ph0_idx]
                else:
                    pre = [ph0, ph1a, ph1b]
                return pre, ph2, ph3

            # chunk schedule: the LAST block's pre-final chunks stream FIRST
            # (their granule chains resolve under the other blocks'
            # streaming), then the other blocks, then the final granule's
            # chunks - so the only post-stream work is the final resolve
            tpc_v = var.get("TPC", TPC_CHUNK)

            def blk_chunks(b, lo, hi):
                t0b = b * P
                out = []
                for t0 in range(t0b + lo, t0b + hi, tpc_v):
                    tn = min(tpc_v, t0b + hi - t0)
                    out.append((b, t0, tn))
                return out

            lb = n_blk - 1
            nt_lb = min(P, ta - lb * P)
            if var.get("ILV", True):
                # last block's body first, then the other blocks' bodies,
                # then the tail ladder: the other blocks' small tracked
                # tails, and the final granule's chunks at the very end
                chunks = blk_chunks(lb, 0, blk_ft_lo[lb])
                for b in range(n_blk - 1):
                    chunks += blk_chunks(b, 0, blk_ft_lo[b])
                for b in range(n_blk - 1):
                    chunks += blk_chunks(b, blk_ft_lo[b], min(P, ta - b * P))
                chunks += blk_chunks(lb, blk_ft_lo[lb], nt_lb)
            else:
                chunks = []
                for b in range(n_blk):
                    chunks += blk_chunks(b, 0, min(P, ta - b * P))
            ci_last = len(chunks) - 1

            def emit_body():
                if tp and not var.get("SKIP_TAILS"):
                    if psg_end:
                        psg_load()
                    else:
                        psg_phase()
                blk_tiles = {}
                pending = []         # (due_chunk_idx, phase_fn)
                deferred = []        # (is_final, ph2, ph3)
                gran_seen = set()
                for ci, (b, t0, tn) in enumerate(chunks):
                    t0b = b * P
                    nt = min(P, ta - t0b)
                    plan = blk_plan[b]
                    trk_base = blk_trk_base[b]
                    trk_wlo = blk_trk_wlo[b]
                    if b not in blk_tiles:
                        mx = stats.tile([P, P], f32, tag=f"mx{b}")
                        if nt < P:
                            nc.vector.memset(mx[:], NEG)
                        ix = None
                        if trk_base is not None:
                            # 8-wide per token: max_index outputs land in
                            # place; covers tokens [trk_base, trk_base+ixw)
                            # (cols below the tracked region stay zero and
                            # only feed redundant resolve rows)
                            ixw = 0
                            for lo2, hi2, tr in plan:
                                if tr:
                                    gt2 = ceil32(hi2 - lo2)
                                    base2 = lo2 if lo2 + gt2 <= P else P - gt2
                                    ixw = max(ixw, base2 + gt2 - trk_base)
                            ixw = min(ixw, P - trk_base)
                            ix = stats.tile([P, ixw * 8], u32, tag=f"ix{b}")
                            nc.vector.memset(ix[:], 0)
                        blk_tiles[b] = (mx, ix)
                    mx, ix = blk_tiles[b]
                    cols = tn * C
                    lt = lpool.tile([P, tpc_v * C], f32, tag="lt")
                    ldeng = nc.scalar if (var.get("DUAL_HWDGE") and ci % 2) else nc.sync
                    ldeng.dma_start(out=lt[:, 0:cols],
                                    in_=lg2[:, t0 * C:(t0 + tn) * C])
                    if not var.get("SKIP_ACCUM"):
                        # CCE-add descriptors must stay <= 2048 elements on HW:
                        # split the accumulate at 8-token granularity
                        for a0 in range(0, tn, TPC):
                            an = min(TPC, tn - a0)
                            nc.gpsimd.dma_start(
                                out=lt[:, a0 * C:(a0 + an) * C],
                                in_=gm2[:, (t0 + a0) * C:(t0 + a0 + an) * C],
                                accum_op=Op.add)
                    if not var.get("SKIP_REDUCE"):
                        col = t0 - t0b
                        # all tn per-token strip maxes in ONE 3D-AP reduce
                        nc.vector.reduce_max(
                            out=mx[:, col:col + tn],
                            in_=lt[:, 0:cols].rearrange("p (t c) -> p t c", c=C),
                            axis=AX.X)
                        if trk_base is not None:
                            # within-strip argmax per token, tracked in-stream
                            # for tokens in the tracked suffix; max_index
                            # writes its 8-wide result straight into ix
                            for j in range(tn):
                                if col + j >= trk_wlo:
                                    jj = col + j - trk_base
                                    nc.vector.max_index(
                                        out=ix[:, jj * 8:(jj + 1) * 8],
                                        in_max=mx[:, col + j:col + j + 1]
                                            .to_broadcast([P, 8]),
                                        in_values=lt[:, j * C:(j + 1) * C])
                    elif t0 + tn >= t0b + nt:
                        nc.vector.reduce_max(out=mx[:, 0:1],
                                             in_=lt[:, 0:C], axis=AX.X)
                    if var.get("SKIP_TAILS"):
                        continue
                    # queue tail phases for granules completed by this chunk;
                    # dues are spread so each phase's inputs are long-ready
                    # when the in-order queues reach them, and rescaled into
                    # the remaining-chunk window so no non-final granule's
                    # chain leaks past the stream end
                    streamed = t0 - t0b + tn
                    for (lo, hi, tracked) in plan:
                        if (b, lo) in gran_seen or streamed < hi:
                            continue
                        gran_seen.add((b, lo))
                        final = tracked and hi == nt and b == n_blk - 1
                        if final:
                            dues = (0, 1, 2, 3)
                        else:
                            dues = var.get("PH_DUES_T", (0, 2, 4, 6)) if tracked \
                                else var.get("PH_DUES", (0, 10, 20, 30, 40))
                            # compress into the remaining-chunk window (minus
                            # a reserve kept clear for the final resolve) so
                            # no granule's chain leaks past the stream end
                            win = ci_last - ci - reserve
                            if dues[-1] > win > 0:
                                sc = win / dues[-1]
                                dues = tuple(int(round(d * sc)) for d in dues)
                        ladder = (tracked and hi == nt
                                  and (not final or var.get("POOL_FINAL"))
                                  and var.get("POOL_LADDER", True))
                        pre, ph2, ph3 = granule_phases(b, lo, hi, tracked,
                                                       mx, ix, trk_base,
                                                       use_pool=ladder)
                        while len(dues) < len(pre) + 2:
                            dues = dues + (dues[-1] + 2,)
                        for k, ph in enumerate(pre):
                            pending.append((ci + dues[k], ph))
                        if defer:
                            # W gather + store run in the end window where
                            # the DMA pool is otherwise idle; the final
                            # granule's pair is emitted LAST
                            deferred.append((final, ph2, ph3))
                        else:
                            for k, ph in enumerate((ph2, ph3)):
                                pending.append((ci + dues[len(pre) + k], ph))
                    # emit everything due after this chunk, in phase order
                    due = [x for x in pending if x[0] <= ci]
                    pending = [x for x in pending if x[0] > ci]
                    for _, ph in due:
                        ph()
                if var.get("SKIP_FLUSH"):
                    pending = []
                    deferred.clear()
                # ---- end flush ----
                # order: non-final W gathers (Pool desc-gens pipeline while
                # the final DVE resolve runs, transfers fill the DMA pool),
                # psg gather, remaining pre phases (final ph0_idx), final W
                # gather, then the stores (scalar ring) with the final store
                # on the idle sync ring
                if var.get("POOL_FINAL"):
                    for fin, ph2, ph3 in deferred:
                        if not fin:
                            ph2()
                    if tp and psg_end and not var.get("SKIP_TAILS"):
                        psg_gather()
                    for _, ph in sorted(pending, key=lambda x: x[0]):
                        ph()
                    for fin, ph2, ph3 in deferred:
                        if fin:
                            ph2()
                else:
                    for _, ph in sorted(pending, key=lambda x: x[0]):
                        ph()
                    for fin, ph2, ph3 in deferred:
                        if not fin:
                            ph2()
                    if tp and psg_end and not var.get("SKIP_TAILS"):
                        psg_gather()
                    for fin, ph2, ph3 in deferred:
                        if fin:
                            ph2()
                for fin, ph2, ph3 in deferred:
                    if not fin:
                        ph3()
                if tp and psg_end and not var.get("SKIP_TAILS"):
                    psg_store()
                for fin, ph2, ph3 in deferred:
                    if fin:
                        ph3()

            emit_body()

    return nc


_BUILD_CACHE = {}


def _get_module(ta, tp, variant=None):
    key = (ta, tp, tuple(sorted((variant or {}).items())))
    if key not in _BUILD_CACHE:
        import concourse.bacc as bacc

        nc = bacc.Bacc("TRN2", target_bir_lowering=False, debug=False)
        _build(nc, ta, tp, variant)
        nc.compile()
        _BUILD_CACHE[key] = nc
    return _BUILD_CACHE[key]


# The reference's f32 grid_sample-nearest index maps, precomputed with jnp
# (the backend the reference runs on) for the hardcoded V=32128 / E=768:
# the column map is exactly identity; the row map is identity except at
# these 17 indices (f32 rounding of the normalized-coordinate roundtrip).
_ROWMAP_DIFF_IDX = [1, 2, 6, 11, 16, 32079, 32089, 32093, 32099, 32103,
                    32107, 32109, 32113, 32117, 32119, 32121, 32123]
_ROWMAP_DIFF_VAL = [0, 1, 5, 10, 15, 32080, 32090, 32094, 32100, 32104,
                    32108, 32110, 32114, 32118, 32120, 32122, 32124]


def _nearest_maps():
    rowmap = np.arange(V, dtype=np.int32)
    rowmap[_ROWMAP_DIFF_IDX] = _ROWMAP_DIFF_VAL
    return rowmap, np.arange(E, dtype=np.int32)


# test/dev hooks: set TRACE=True before calling kernel() to capture an NTFF
# profile; the BassKernelResults of the last run is stored in LAST_RESULT.
TRACE = False
LAST_RESULT = None
LAST_MODULE = None
DEFAULT_VARIANT = None   # dev hook: build-variant dict used by kernel()


def _fold(rows):
    """[n, V] f32 -> [128*n, 251] partition-major fold."""
    n = rows.shape[0]
    return np.ascontiguousarray(
        rows.reshape(n, P, C).transpose(1, 0, 2).reshape(P * n, C))


def kernel(logits, rwrt_attention_mask, psg_input_ids, word_embeddings, gumbel_noise):
    from concourse.bass_utils import run_bass_kernel_spmd

    logits = np.ascontiguousarray(np.asarray(logits, dtype=np.float32)).reshape(B * L, V)
    gumbel = np.ascontiguousarray(np.asarray(gumbel_noise, dtype=np.float32)).reshape(B * L, V)
    mask = np.asarray(rwrt_attention_mask, dtype=np.int32)
    psg = np.asarray(psg_input_ids, dtype=np.int32)
    wte = np.ascontiguousarray(np.asarray(word_embeddings, dtype=np.float32))

    # wrm = W[rowmap] precomposed (rowmap is identity except 17 rows)
    wrm = wte.copy()
    wrm[_ROWMAP_DIFF_IDX] = wte[_ROWMAP_DIFF_VAL]

    lens = mask.sum(axis=1)
    contiguous = bool(np.all(mask == (np.arange(L)[None, :] < lens[:, None])))

    if contiguous:
        # fast path: the two branches are positionally disjoint (see header)
        act_pos = []           # flat b*L+l, in output order
        psg_pos = []
        psg_rows = []
        for b in range(B):
            ln = int(lens[b])
            act_pos.extend(b * L + l for l in range(ln))
            for l in range(ln, L):
                psg_pos.append(b * L + l)
                psg_rows.append(1 if l == ln else int(psg[b, l - ln - 1]))
    else:
        # general fallback (never taken for the reference's inputs): stream
        # every position's argmax on device, gather both branches' W rows on
        # device, combine per the reference's mask/flag weights at unshard
        # time. Index arithmetic below mirrors the reference exactly.
        act_pos = list(range(B * L))
        psg_roll = np.roll(psg, 1, axis=1)
        psg_roll[:, 0] = 1
        extr = (1 - mask[:, ::-1]) * psg_roll
        pos = (np.arange(L)[None, :] - lens[:, None]) % L
        trunc = np.take_along_axis(extr, pos, axis=1)
        flag = (np.cumsum(trunc != 0, axis=1) > 0).astype(np.float32)
        psg_pos = list(range(B * L))
        psg_rows = trunc.reshape(-1).tolist()
    A, Pn = len(act_pos), len(psg_pos)
    ta = max(1, (A + N_CORES - 1) // N_CORES)
    tp = (Pn + N_CORES - 1) // N_CORES

    pad_src = act_pos[-1] if act_pos else 0
    act_idx = np.asarray(act_pos + [pad_src] * (ta * N_CORES - A), np.int64)
    psg_idx = np.asarray(psg_rows + [0] * (tp * N_CORES - Pn), np.int32)

    nc = _get_module(ta, tp, DEFAULT_VARIANT)
    global LAST_MODULE
    LAST_MODULE = nc

    in_maps = []
    for m in range(N_CORES):
        sl = act_idx[m * ta:(m + 1) * ta]
        im = {
            "lgg": np.concatenate([_fold(logits[sl]), _fold(gumbel[sl])], axis=0),
            "wrm": wrm,
        }
        if tp:
            im["wte"] = wte
            im["psgidx"] = np.ascontiguousarray(
                psg_idx[m * tp:(m + 1) * tp].reshape(tp, 1))
        in_maps.append(im)

    global LAST_RESULT
    try:
        LAST_RESULT = run_bass_kernel_spmd(nc, in_maps, list(range(N_CORES)), trace=TRACE)
    except Exception:
        # the axon-relayed device occasionally reports a transient
        # NRT_EXEC_UNIT_UNRECOVERABLE on the first execution after long
        # sessions; a straight re-run recovers it
        import time as _time

        _time.sleep(2.0)
        LAST_RESULT = run_bass_kernel_spmd(nc, in_maps, list(range(N_CORES)), trace=TRACE)
    res = LAST_RESULT.results

    acts = np.concatenate([res[m]["out_act"] for m in range(N_CORES)], axis=0)
    if contiguous:
        out = np.empty((B * L, E), np.float32)
        out[np.asarray(act_pos, np.int64)] = acts[:A]
        if Pn:
            psgs = np.concatenate(
                [res[m]["out_psg"] for m in range(N_CORES)], axis=0)
            out[np.asarray(psg_pos, np.int64)] = psgs[:Pn]
        return out.reshape(B, L, E)
    psgs = np.concatenate([res[m]["out_psg"] for m in range(N_CORES)], axis=0)
    out = (acts[:A] * mask.reshape(-1, 1)
           + psgs[:Pn] * flag.reshape(-1, 1)).astype(np.float32)
    return out.reshape(B, L, E)
